# revision 2
# baseline (speedup 1.0000x reference)
"""Trainium2 Bass kernel for batched additive-attention scoring.

Computes, for each batch b:
    out[b] = softmax_s( sum_h v[h] * tanh( (W1 @ static[b])[h,s]
                                         + (W2 @ dynamic[b])[h,s]
                                         + (W3 @ hidden[b])[h] ) )

Sharding: data-parallel over batch B=64 across 8 NeuronCores (8 batches
per core); small params (W, v) replicated.  No collectives needed.

Per-core dataflow (H=256, S=4096):
  - encoders are cast to fp16 and concatenated on host: 32 MB/core of
    DMA; fp16 matmuls run at 1 col/cycle (216 ns per N=512 at 2.37 GHz),
    PE floor = 576x216 = 124.5 us; quantization error ~7e-4 vs the 2e-2
    gate.
  - head: the 8 critical first DMAs (4 wt chunks + 4 x 512-col first
    slices) are split across the sync and scalar HWDGE rings so issue
    overhead (~590 ns each) parallelizes; group 0 runs j-major (all 4
    k-chunks of s-cols 0:512 first) so the PE starts on 0.7 MB of data
    instead of 1.3 MB.  Batch b+1's x chunks (2 per ring) are issued at
    the top of batch b (xpool bufs=16 = 4 batches) so batch-boundary
    matmuls never wait on DMA.
  - per (batch, s-pair): 8 E-matmuls (N=512, PSUM-bank cap) accumulate
    K=512 into the two halves of a [128, 2, 512] two-bank psum tile; ONE
    tanh reads the flat [128, 1024] view (per-partition bias, fp16 out).
  - DVE combines the two m-chunk tanh outputs in 2 ops (tensor_scalar
    mul + scalar_tensor_tensor mult-add): esc = v0*E0 + v1*E1.
  - the v-matmuls (ones-window partition sum into score row r) run THREE
    groups late so they never wait on the tanh->DVE chain.
  - scores accumulate in TWO psum banks (batches 0-3 | 4-7); per half:
    one Exp [32,512] with accum_out; per-batch totals via ONE block-ones
    fp16 matmul that lands Z directly on each score row's partition
    (b2[p,r]=1 iff same batch); f32 DVE reciprocal; DVE scale; 64 KB
    output DMA on the scalar ring.  Half A runs mid-kernel; only half B
    sits on the tail.
"""

import os
import sys
from contextlib import ExitStack

import numpy as np

for _p in ("/root/.axon_site", "/root/.axon_site/_ro/trn_rl_repo",
           "/root/.axon_site/_ro/pypackages", "/opt/trn_rl_repo", "/opt/pypackages"):
    if os.path.isdir(_p) and _p not in sys.path:
        sys.path.append(_p)

import concourse.bass as bass
import concourse.tile as tile
from concourse import bacc, mybir
from concourse._compat import with_exitstack
from concourse.bass_utils import run_bass_kernel_spmd

H = 256
S = 4096
B = 64
NCORES = 8
BPC = B // NCORES  # batches per core

F32 = mybir.dt.float32
F32R = mybir.dt.float32r
F16 = mybir.dt.float16
TANH = mybir.ActivationFunctionType.Tanh
EXP = mybir.ActivationFunctionType.Exp
MULT = mybir.AluOpType.mult
ADD = mybir.AluOpType.add

ST = 512           # matmul output tile (one PSUM bank of f32, ISA cap)
NS = S // ST       # 8 s-tiles
NG = NS // 2       # 4 s-pairs per batch
NM = H // 128      # 2 m-blocks (output h partition blocks)
NK = (2 * H) // 128  # 4 k-chunks of the concatenated [static; dynamic]
NROW = BPC * NS    # 64 score rows (one per (batch, s-tile))
HROW = NROW // 2   # rows per scores half-bank
VLAG = 3           # groups the v-matmuls run behind the tanh/DVE chain


@with_exitstack
def _attn_kernel(ctx: ExitStack, tc: "tile.TileContext",
                 out_ap, x_ap, wt_ap, w3t_ap, vt_ap, ht_ap, b2_ap, vp_ap):
    nc = tc.nc

    const = ctx.enter_context(tc.tile_pool(name="const", bufs=1))
    xpool = ctx.enter_context(tc.tile_pool(name="x", bufs=16))
    epsum = ctx.enter_context(tc.tile_pool(name="epsum", bufs=3, space="PSUM"))
    scpsum = ctx.enter_context(tc.tile_pool(name="scpsum", bufs=1, space="PSUM"))
    esb = ctx.enter_context(tc.tile_pool(name="esb", bufs=8))
    ecb = ctx.enter_context(tc.tile_pool(name="ecb", bufs=8))
    rows = ctx.enter_context(tc.tile_pool(name="rows", bufs=1))
    tiny = ctx.enter_context(tc.tile_pool(name="tiny", bufs=4))

    # ---- head: the first E-matmul group (b0, g0) runs j-major, so it
    # needs only wt + the 512-col first slice of each k-chunk.  Interleave
    # those 8 DMAs across the two HWDGE rings (sync + scalar) so their
    # ~590 ns issue costs overlap; each ring's preamble ends ~6.5 us in.
    wt_sb = const.tile([128, NK, H], F16)        # [p, kchunk, h]
    xt0 = []
    for c in range(NK):
        t = xpool.tile([128, S], F16, tag="x", name=f"x{c}")
        xt0.append(t)
    for c in range(NK):
        nc.sync.dma_start(wt_sb[:, c, :], wt_ap[:, c, :])
        nc.scalar.dma_start(xt0[c][:, 0:ST], x_ap[0, c * 128:(c + 1) * 128, 0:ST])
    # second 512-col slices (j=1 of group 0), alternating rings
    for c in range(NK):
        eng = nc.sync if c % 2 == 0 else nc.scalar
        eng.dma_start(xt0[c][:, ST:2 * ST],
                      x_ap[0, c * 128:(c + 1) * 128, ST:2 * ST])

    # replicated params on the gpsimd (SWDGE) ring: bias inputs needed
    # ~12 us in, the rest later
    w3_sb = const.tile([128, 2, H], F32R)        # [p, kchunk, h]
    nc.gpsimd.dma_start(w3_sb[:], w3t_ap)
    ht_sb = const.tile([128, 2, BPC], F32R)      # [p, kchunk, b]
    nc.gpsimd.dma_start(ht_sb[:], ht_ap)
    vt_sb = const.tile([128, 2, 2 * HROW - 1], F16)  # ones window, 0-padded
    nc.gpsimd.dma_start(vt_sb[:], vt_ap)
    b2_sb = const.tile([128, NROW], F16)         # b2[p,r]=1 iff same batch
    nc.gpsimd.dma_start(b2_sb[:], b2_ap)
    vp_sb = const.tile([128, 2], F32)            # v chunks, per-partition
    nc.gpsimd.dma_start(vp_sb[:], vp_ap)

    # rest of batch 0 (cols 1024:4096) as 1024-col quarters, c-major
    # interleaved across rings
    for q in range(1, 4):
        for c in range(NK):
            eng = nc.sync if c % 2 == 0 else nc.scalar
            eng.dma_start(xt0[c][:, q * 1024:(q + 1) * 1024],
                          x_ap[0, c * 128:(c + 1) * 128, q * 1024:(q + 1) * 1024])

    # two psum banks accumulating score rows (batches 0-3 | 4-7) so the
    # first half's exp runs mid-kernel instead of on the tail
    scores_box = [None, None]

    # ---- bias[h, b] = sum_k W3T[k,h] * hiddenT[k,b] (all batches at once).
    # Emitted AFTER batch 0's first j-block: the in-order PE sequencer
    # would otherwise stall on the (slow SWDGE) w3/ht loads before
    # dispatching any E-matmul.
    bias_sb = const.tile([128, NM, BPC], F32)  # [p, m, b]

    def emit_bias():
        for m in range(NM):
            bp = scpsum.tile([128, BPC], F32, tag="scA", name="bp")
            for c in range(2):
                nc.tensor.matmul(bp[:],
                                 lhsT=w3_sb[:, c, m * 128:(m + 1) * 128],
                                 rhs=ht_sb[:, c, :],
                                 start=(c == 0), stop=(c == 1))
            nc.vector.tensor_copy(bias_sb[:, m, :], bp[:])

    exp_sb = rows.tile([NROW, ST], F32, tag="exp")
    sums = tiny.tile([128, 2], F16, tag="sums")
    first_v = [True, True]

    inv32 = tiny.tile([NROW, 1], F32, tag="inv32")

    def emit_exp(h):
        # exp one half of the score rows; h=0 runs mid-kernel (hidden
        # behind batches 4-7), h=1 on the tail
        with nc.allow_low_precision(reason="fp16 denominators, ~5e-4 rel"):
            nc.scalar.activation(
                exp_sb[h * HROW:(h + 1) * HROW, :], scores_box[h][:], EXP,
                accum_out=sums[h * HROW:(h + 1) * HROW, 0:1])

    def emit_normalize(h):
        # per-batch totals via ONE block-ones matmul landing Z_b on every
        # score row's partition, f32 reciprocal, scale, output DMA
        p0 = h * HROW
        tot = scpsum.tile([NROW, 2], F32, tag="scA", name=f"tot{h}")
        nc.tensor.matmul(tot[p0:p0 + HROW, :],
                         lhsT=b2_sb[p0:p0 + HROW, p0:p0 + HROW],
                         rhs=sums[p0:p0 + HROW, :], start=True, stop=True)
        nc.vector.reciprocal(inv32[p0:p0 + HROW, :], tot[p0:p0 + HROW, 0:1])
        nc.vector.tensor_scalar_mul(exp_sb[p0:p0 + HROW, :],
                                    exp_sb[p0:p0 + HROW, :],
                                    inv32[p0:p0 + HROW, :])
        nc.scalar.dma_start(out_ap[h * (BPC // 2):(h + 1) * (BPC // 2), :],
                            exp_sb[p0:p0 + HROW, :])

    def emit_v(pend):
        # one ones-window matmul per s-tile: the v-weighting already
        # happened on the DVE (esc = v0*E0 + v1*E1), so the matmul is
        # a plain partition sum into score row r
        r2, esc = pend
        h, hr2 = r2 // HROW, r2 % HROW
        scores = scores_box[h]
        for j in range(2):
            r = hr2 + j
            nc.tensor.matmul(
                scores[:],
                lhsT=vt_sb[:, 0, (HROW - 1) - r:(HROW - 1) - r + HROW],
                rhs=esc[:, j, :],
                start=first_v[h],
                stop=(r == HROW - 1),
                skip_group_check=True)
            first_v[h] = False
        if r2 + 2 == HROW:
            emit_exp(0)
        elif r2 == HROW:
            emit_normalize(0)

    pending = []
    for b in range(BPC):
        if b == 0:
            xt = xt0
        else:
            xt = xt_next  # noqa: F821  (set on the previous iteration)
        if b + 1 < BPC:
            # prefetch batch b+1's 4 k-chunks as full [128, 4096] DMAs
            # (8 KB descriptor rows), 2 per HWDGE ring
            xt_next = []
            for c in range(NK):
                t = xpool.tile([128, S], F16, tag="x", name=f"x{c}")
                xt_next.append(t)
            for c in range(NK):
                eng = nc.sync if c < 2 else nc.scalar
                eng.dma_start(xt_next[c][:],
                              x_ap[b + 1, c * 128:(c + 1) * 128, :])

        for g in range(NG):
            eps_m = []
            for m in range(NM):
                eps_m.append(epsum.tile([128, 2, ST], F32, tag="ep",
                                        name=f"ep{m}"))
            if b == 0 and g == 0:
                # j-major so the group starts on the 512-col first slices
                for j in range(2):
                    for m in range(NM):
                        for c in range(NK):
                            nc.tensor.matmul(
                                eps_m[m][:, j, :],
                                lhsT=wt_sb[:, c, m * 128:(m + 1) * 128],
                                rhs=xt[c][:, j * ST:(j + 1) * ST],
                                start=(c == 0), stop=(c == NK - 1))
                    if j == 0:
                        # bias matmuls ride here: after the first j-block
                        # (so the PE has work while the SWDGE w3/ht loads
                        # land) but before the first tanh reads bias_sb
                        emit_bias()
                        scores_box[0] = scpsum.tile([HROW, ST], F32,
                                                    tag="scA", name="scoresA")
                        scores_box[1] = scpsum.tile([HROW, ST], F32,
                                                    tag="scB", name="scoresB")
                        nc.vector.memset(sums[:], 0.0)
            else:
                for m in range(NM):
                    for c in range(NK):
                        for j in range(2):
                            nc.tensor.matmul(
                                eps_m[m][:, j, :],
                                lhsT=wt_sb[:, c, m * 128:(m + 1) * 128],
                                rhs=xt[c][:, (2 * g + j) * ST:(2 * g + j + 1) * ST],
                                start=(c == 0), stop=(c == NK - 1))
            es_pair = []
            for m in range(NM):
                es = esb.tile([128, 2, ST], F16, tag="es")
                nc.scalar.activation(es[:], eps_m[m][:],
                                     TANH, bias=bias_sb[:, m, b:b + 1])
                es_pair.append(es)

            esc = ecb.tile([128, 2, ST], F16, tag="ec")
            tmp = ecb.tile([128, 2, ST], F16, tag="ec2")
            nc.vector.tensor_scalar_mul(tmp[:], es_pair[1][:], vp_sb[:, 1:2])
            nc.vector.scalar_tensor_tensor(esc[:], es_pair[0][:],
                                           vp_sb[:, 0:1], tmp[:],
                                           op0=MULT, op1=ADD)
            pending.append((b * NS + 2 * g, esc))
            if len(pending) > VLAG:
                emit_v(pending.pop(0))

    for pend in pending:
        emit_v(pend)

    # ---- tail: exp + normalize + output DMA for half B only (half A
    # already went out mid-kernel)
    emit_exp(1)
    emit_normalize(1)


_CACHED = None


def _build():
    global _CACHED
    if _CACHED is not None:
        return _CACHED
    nc = bacc.Bacc("TRN2", target_bir_lowering=False, debug=False,
                   num_devices=NCORES)
    x = nc.dram_tensor("x", (BPC, 2 * H, S), F16, kind="ExternalInput").ap()
    wt = nc.dram_tensor("wt", (128, NK, H), F16, kind="ExternalInput").ap()
    w3t = nc.dram_tensor("w3t", (128, 2, H), F32R, kind="ExternalInput").ap()
    vt = nc.dram_tensor("vt", (128, 2, 2 * HROW - 1), F16, kind="ExternalInput").ap()
    ht = nc.dram_tensor("ht", (128, 2, BPC), F32R, kind="ExternalInput").ap()
    b2 = nc.dram_tensor("b2", (128, NROW), F16, kind="ExternalInput").ap()
    vp = nc.dram_tensor("vp", (128, 2), F32, kind="ExternalInput").ap()
    out = nc.dram_tensor("out", (BPC, S), F32, kind="ExternalOutput").ap()

    with tile.TileContext(nc) as tc:
        _attn_kernel(tc, out, x, wt, w3t, vt, ht, b2, vp)
    nc.compile()
    _CACHED = nc
    return nc


def _chunk_major(a: np.ndarray) -> np.ndarray:
    """[C*128, F] -> [128, C, F] so partition p holds rows {p, 128+p, ...}."""
    c = a.shape[0] // 128
    return np.ascontiguousarray(a.reshape(c, 128, -1).transpose(1, 0, 2))


def kernel(static_enc, dynamic_enc, decoder_hidden, v, W, *, _trace=False,
           **trace_kwargs):
    static_enc = np.asarray(static_enc, dtype=np.float16)
    dynamic_enc = np.asarray(dynamic_enc, dtype=np.float16)
    decoder_hidden = np.ascontiguousarray(decoder_hidden, dtype=np.float32)
    v = np.ascontiguousarray(v, dtype=np.float32)
    W = np.ascontiguousarray(W, dtype=np.float32)

    nc = _build()

    xcat = np.concatenate([static_enc, dynamic_enc], axis=1)  # [B, 2H, S]
    wt = _chunk_major(np.concatenate([W[:, :H].T, W[:, H:2 * H].T],
                                     axis=0).astype(np.float16))
    w3t = _chunk_major(np.ascontiguousarray(W[:, 2 * H:].T))
    # vt_ext[p, c, :] = [0]*31 ++ [1] ++ [0]*31 ; lhsT window starting
    # at (HROW-1)-r puts the sum at output row r, zeros elsewhere.
    vt_ext = np.zeros((128, 2, 2 * HROW - 1), dtype=np.float16)
    vt_ext[:, :, HROW - 1] = 1.0  # ones window: plain partition sum
    vp = np.ascontiguousarray(v.reshape(2, 128).T.astype(np.float32))
    # b2[p, r] = 1 iff score rows p and r belong to the same batch (both
    # within the same half); the tot matmul then lands Z_b on every score
    # row partition of batch b directly.
    b2 = np.zeros((128, NROW), dtype=np.float16)
    for r in range(NROW):
        for p in range(NROW):
            if p // NS == r // NS:
                b2[p, r] = 1.0
    in_maps = []
    for i in range(NCORES):
        sl = slice(i * BPC, (i + 1) * BPC)
        ht = _chunk_major(np.ascontiguousarray(decoder_hidden[sl].T))
        in_maps.append({
            "x": xcat[sl],
            "wt": wt, "w3t": w3t, "vt": vt_ext, "ht": ht,
            "b2": b2, "vp": vp,
        })

    res = run_bass_kernel_spmd(nc, in_maps, core_ids=list(range(NCORES)),
                               trace=_trace, **trace_kwargs)
    kernel.last_result = res
    return np.concatenate([res.results[i]["out"] for i in range(NCORES)], axis=0)


kernel.last_result = None


# revision 5
# speedup vs baseline: 1.0302x; 1.0302x over previous
"""Trainium2 Bass kernel for batched additive-attention scoring.

Computes, for each batch b:
    out[b] = softmax_s( sum_h v[h] * tanh( (W1 @ static[b])[h,s]
                                         + (W2 @ dynamic[b])[h,s]
                                         + (W3 @ hidden[b])[h] ) )

Sharding: data-parallel over batch B=64 across 8 NeuronCores (8 batches
per core); small params (W, v) replicated.  No collectives needed.

Per-core dataflow (H=256, S=4096):
  - encoders are cast to fp16 and concatenated on host: 32 MB/core of
    DMA; fp16 matmuls run at 1 col/cycle (216 ns per N=512 at 2.37 GHz),
    PE floor = 576x216 = 124.5 us; quantization error ~7e-4 vs the 2e-2
    gate.
  - head: the 8 critical first DMAs (4 wt chunks + 4 x 512-col first
    slices) are split across the sync and scalar HWDGE rings so issue
    overhead (~590 ns each) parallelizes; group 0 runs j-major (all 4
    k-chunks of s-cols 0:512 first) so the PE starts on 0.7 MB of data
    instead of 1.3 MB.  Batch b+1's x chunks (2 per ring) are issued at
    the top of batch b (xpool bufs=16 = 4 batches) so batch-boundary
    matmuls never wait on DMA.
  - per (batch, s-pair): 8 E-matmuls (N=512, PSUM-bank cap) accumulate
    K=512 into the two halves of a [128, 2, 512] two-bank psum tile; ONE
    tanh reads the flat [128, 1024] view (per-partition bias, fp16 out).
  - DVE combines the two m-chunk tanh outputs in 2 ops (tensor_scalar
    mul + scalar_tensor_tensor mult-add): esc = v0*E0 + v1*E1.
  - the v-matmuls (ones-window partition sum into score row r) run THREE
    groups late so they never wait on the tanh->DVE chain.
  - scores accumulate in TWO psum banks (batches 0-3 | 4-7); per half:
    one Exp [32,512] with accum_out; per-batch totals via ONE block-ones
    fp16 matmul that lands Z directly on each score row's partition
    (b2[p,r]=1 iff same batch); f32 DVE reciprocal; DVE scale; 64 KB
    output DMA on the scalar ring.  Half A runs mid-kernel; only half B
    sits on the tail.
"""

import os
import sys
from contextlib import ExitStack

import numpy as np

for _p in ("/root/.axon_site", "/root/.axon_site/_ro/trn_rl_repo",
           "/root/.axon_site/_ro/pypackages", "/opt/trn_rl_repo", "/opt/pypackages"):
    if os.path.isdir(_p) and _p not in sys.path:
        sys.path.append(_p)

import concourse.bass as bass
import concourse.tile as tile
from concourse import bacc, mybir
from concourse._compat import with_exitstack
from concourse.bass_utils import run_bass_kernel_spmd

H = 256
S = 4096
B = 64
NCORES = 8
BPC = B // NCORES  # batches per core

F32 = mybir.dt.float32
F32R = mybir.dt.float32r
F16 = mybir.dt.float16
TANH = mybir.ActivationFunctionType.Tanh
EXP = mybir.ActivationFunctionType.Exp
MULT = mybir.AluOpType.mult
ADD = mybir.AluOpType.add

ST = 512           # matmul output tile (one PSUM bank of f32, ISA cap)
NS = S // ST       # 8 s-tiles
NG = NS // 2       # 4 s-pairs per batch
NM = H // 128      # 2 m-blocks (output h partition blocks)
NK = (2 * H) // 128  # 4 k-chunks of the concatenated [static; dynamic]
NROW = BPC * NS    # 64 score rows (one per (batch, s-tile))
HROW = NROW // 2   # rows per scores half-bank
VLAG = 3           # groups the v-matmuls run behind the tanh/DVE chain


@with_exitstack
def _attn_kernel(ctx: ExitStack, tc: "tile.TileContext",
                 out_ap, x_ap, wt_ap, w3t_ap, vt_ap, ht_ap, b2_ap, vp_ap):
    nc = tc.nc

    const = ctx.enter_context(tc.tile_pool(name="const", bufs=1))
    xpool = ctx.enter_context(tc.tile_pool(name="x", bufs=16))
    epsum = ctx.enter_context(tc.tile_pool(name="epsum", bufs=3, space="PSUM"))
    scpsum = ctx.enter_context(tc.tile_pool(name="scpsum", bufs=1, space="PSUM"))
    esb = ctx.enter_context(tc.tile_pool(name="esb", bufs=8))
    ecb = ctx.enter_context(tc.tile_pool(name="ecb", bufs=8))
    rows = ctx.enter_context(tc.tile_pool(name="rows", bufs=1))
    tiny = ctx.enter_context(tc.tile_pool(name="tiny", bufs=4))

    # ---- head: the first E-matmul group (b0, g0) runs j-major, so it
    # needs only wt + the 512-col first slice of each k-chunk.  Split the
    # head DMAs across the two HWDGE rings (sync + scalar) so their
    # ~590 ns issue costs overlap.  CRITICAL: all DMAs into a given tile
    # must stay on ONE ring — cross-ring writes to a tile make the
    # framework serialize each issue on the previous DMA's completion
    # semaphore (measured 4+ us stalls).  Chunk c rides ring (c mod 2).
    wt_sb = const.tile([128, NK, H], F16)        # [p, kchunk, h]
    nc.sync.dma_start(wt_sb[:], wt_ap)           # one 256 KB DMA, ~0.7 us
    xt0 = []
    for c in range(NK):
        t = xpool.tile([128, S], F16, tag="x", name=f"x{c}")
        xt0.append(t)

    def _ring(c):
        return nc.sync if c % 2 == 0 else nc.scalar

    for j in range(2):  # 512-col slices, j=0 of every chunk before j=1
        for c in range(NK):
            _ring(c).dma_start(xt0[c][:, j * ST:(j + 1) * ST],
                               x_ap[0, c * 128:(c + 1) * 128, j * ST:(j + 1) * ST])

    # replicated params on the gpsimd (SWDGE) ring: bias inputs needed
    # ~12 us in, the rest later
    w3_sb = const.tile([128, 2, H], F32R)        # [p, kchunk, h]
    nc.gpsimd.dma_start(w3_sb[:], w3t_ap)
    ht_sb = const.tile([128, 2, BPC], F32R)      # [p, kchunk, b]
    nc.gpsimd.dma_start(ht_sb[:], ht_ap)
    vt_sb = const.tile([128, 2, 2 * HROW - 1], F16)  # ones window, 0-padded
    nc.gpsimd.dma_start(vt_sb[:], vt_ap)
    b2_sb = const.tile([128, NROW], F16)         # b2[p,r]=1 iff same batch
    nc.gpsimd.dma_start(b2_sb[:], b2_ap)
    vp_sb = const.tile([128, 2], F32)            # v chunks, per-partition
    nc.gpsimd.dma_start(vp_sb[:], vp_ap)

    # rest of batch 0 (cols 1024:4096) as 1024-col quarters, c-major,
    # each chunk staying on its ring
    for q in range(1, 4):
        for c in range(NK):
            _ring(c).dma_start(xt0[c][:, q * 1024:(q + 1) * 1024],
                               x_ap[0, c * 128:(c + 1) * 128, q * 1024:(q + 1) * 1024])

    # two psum banks accumulating score rows (batches 0-3 | 4-7) so the
    # first half's exp runs mid-kernel instead of on the tail
    scores_box = [None, None]

    # ---- bias[h, b] = sum_k W3T[k,h] * hiddenT[k,b] (all batches at once).
    # Emitted AFTER batch 0's first j-block: the in-order PE sequencer
    # would otherwise stall on the (slow SWDGE) w3/ht loads before
    # dispatching any E-matmul.
    bias_sb = const.tile([128, NM, BPC], F32)  # [p, m, b]

    def emit_bias():
        for m in range(NM):
            bp = scpsum.tile([128, BPC], F32, tag="scA", name="bp")
            for c in range(2):
                nc.tensor.matmul(bp[:],
                                 lhsT=w3_sb[:, c, m * 128:(m + 1) * 128],
                                 rhs=ht_sb[:, c, :],
                                 start=(c == 0), stop=(c == 1))
            nc.vector.tensor_copy(bias_sb[:, m, :], bp[:])

    exp_sb = rows.tile([NROW, ST], F32, tag="exp")
    sums = tiny.tile([128, 2], F16, tag="sums")
    first_v = [True, True]

    inv32 = tiny.tile([NROW, 1], F32, tag="inv32")

    def emit_exp(h):
        # exp one half of the score rows; h=0 runs mid-kernel (hidden
        # behind batches 4-7), h=1 on the tail
        with nc.allow_low_precision(reason="fp16 denominators, ~5e-4 rel"):
            nc.scalar.activation(
                exp_sb[h * HROW:(h + 1) * HROW, :], scores_box[h][:], EXP,
                accum_out=sums[h * HROW:(h + 1) * HROW, 0:1])

    def emit_normalize(h):
        # per-batch totals via ONE block-ones matmul landing Z_b on every
        # score row's partition, f32 reciprocal, scale, output DMA
        p0 = h * HROW
        tot = scpsum.tile([NROW, 2], F32, tag="scA", name=f"tot{h}")
        nc.tensor.matmul(tot[p0:p0 + HROW, :],
                         lhsT=b2_sb[p0:p0 + HROW, p0:p0 + HROW],
                         rhs=sums[p0:p0 + HROW, :], start=True, stop=True)
        nc.vector.reciprocal(inv32[p0:p0 + HROW, :], tot[p0:p0 + HROW, 0:1])
        nc.vector.tensor_scalar_mul(exp_sb[p0:p0 + HROW, :],
                                    exp_sb[p0:p0 + HROW, :],
                                    inv32[p0:p0 + HROW, :])
        nc.scalar.dma_start(out_ap[h * (BPC // 2):(h + 1) * (BPC // 2), :],
                            exp_sb[p0:p0 + HROW, :])

    def emit_v(pend):
        # one ones-window matmul per s-tile: the v-weighting already
        # happened on the DVE (esc = v0*E0 + v1*E1), so the matmul is
        # a plain partition sum into score row r
        r2, esc = pend
        h, hr2 = r2 // HROW, r2 % HROW
        scores = scores_box[h]
        for j in range(2):
            r = hr2 + j
            nc.tensor.matmul(
                scores[:],
                lhsT=vt_sb[:, 0, (HROW - 1) - r:(HROW - 1) - r + HROW],
                rhs=esc[:, j, :],
                start=first_v[h],
                stop=(r == HROW - 1),
                skip_group_check=True)
            first_v[h] = False
        if r2 + 2 == HROW:
            emit_exp(0)
        elif r2 == HROW:
            emit_normalize(0)

    pending = []
    for b in range(BPC):
        if b == 0:
            xt = xt0
        else:
            xt = xt_next  # noqa: F821  (set on the previous iteration)
        if b + 1 < BPC:
            # prefetch batch b+1's 4 k-chunks as full [128, 4096] DMAs
            # (8 KB descriptor rows), 2 per HWDGE ring
            xt_next = []
            for c in range(NK):
                t = xpool.tile([128, S], F16, tag="x", name=f"x{c}")
                xt_next.append(t)
            for c in range(NK):
                _ring(c).dma_start(xt_next[c][:],
                                   x_ap[b + 1, c * 128:(c + 1) * 128, :])

        for g in range(NG):
            eps_m = []
            for m in range(NM):
                eps_m.append(epsum.tile([128, 2, ST], F32, tag="ep",
                                        name=f"ep{m}"))
            if b == 0 and g == 0:
                # j-major so the group starts on the 512-col first slices
                for j in range(2):
                    for m in range(NM):
                        for c in range(NK):
                            nc.tensor.matmul(
                                eps_m[m][:, j, :],
                                lhsT=wt_sb[:, c, m * 128:(m + 1) * 128],
                                rhs=xt[c][:, j * ST:(j + 1) * ST],
                                start=(c == 0), stop=(c == NK - 1))
                    if j == 0:
                        # bias matmuls ride here: after the first j-block
                        # (so the PE has work while the SWDGE w3/ht loads
                        # land) but before the first tanh reads bias_sb
                        emit_bias()
                        scores_box[0] = scpsum.tile([HROW, ST], F32,
                                                    tag="scA", name="scoresA")
                        scores_box[1] = scpsum.tile([HROW, ST], F32,
                                                    tag="scB", name="scoresB")
                        nc.vector.memset(sums[:], 0.0)
            else:
                for m in range(NM):
                    for c in range(NK):
                        for j in range(2):
                            nc.tensor.matmul(
                                eps_m[m][:, j, :],
                                lhsT=wt_sb[:, c, m * 128:(m + 1) * 128],
                                rhs=xt[c][:, (2 * g + j) * ST:(2 * g + j + 1) * ST],
                                start=(c == 0), stop=(c == NK - 1))
            es_pair = []
            for m in range(NM):
                es = esb.tile([128, 2, ST], F16, tag="es")
                nc.scalar.activation(es[:], eps_m[m][:],
                                     TANH, bias=bias_sb[:, m, b:b + 1])
                es_pair.append(es)

            esc = ecb.tile([128, 2, ST], F16, tag="ec")
            tmp = ecb.tile([128, 2, ST], F16, tag="ec2")
            nc.vector.tensor_scalar_mul(tmp[:], es_pair[1][:], vp_sb[:, 1:2])
            nc.vector.scalar_tensor_tensor(esc[:], es_pair[0][:],
                                           vp_sb[:, 0:1], tmp[:],
                                           op0=MULT, op1=ADD)
            pending.append((b * NS + 2 * g, esc))
            if len(pending) > VLAG:
                emit_v(pending.pop(0))

    for pend in pending:
        emit_v(pend)

    # ---- tail: exp + normalize + output DMA for half B only (half A
    # already went out mid-kernel)
    emit_exp(1)
    emit_normalize(1)


_CACHED = None


def _build():
    global _CACHED
    if _CACHED is not None:
        return _CACHED
    nc = bacc.Bacc("TRN2", target_bir_lowering=False, debug=False,
                   num_devices=NCORES)
    x = nc.dram_tensor("x", (BPC, 2 * H, S), F16, kind="ExternalInput").ap()
    wt = nc.dram_tensor("wt", (128, NK, H), F16, kind="ExternalInput").ap()
    w3t = nc.dram_tensor("w3t", (128, 2, H), F32R, kind="ExternalInput").ap()
    vt = nc.dram_tensor("vt", (128, 2, 2 * HROW - 1), F16, kind="ExternalInput").ap()
    ht = nc.dram_tensor("ht", (128, 2, BPC), F32R, kind="ExternalInput").ap()
    b2 = nc.dram_tensor("b2", (128, NROW), F16, kind="ExternalInput").ap()
    vp = nc.dram_tensor("vp", (128, 2), F32, kind="ExternalInput").ap()
    out = nc.dram_tensor("out", (BPC, S), F32, kind="ExternalOutput").ap()

    with tile.TileContext(nc) as tc:
        _attn_kernel(tc, out, x, wt, w3t, vt, ht, b2, vp)
    nc.compile()
    _CACHED = nc
    return nc


def _chunk_major(a: np.ndarray) -> np.ndarray:
    """[C*128, F] -> [128, C, F] so partition p holds rows {p, 128+p, ...}."""
    c = a.shape[0] // 128
    return np.ascontiguousarray(a.reshape(c, 128, -1).transpose(1, 0, 2))


def kernel(static_enc, dynamic_enc, decoder_hidden, v, W, *, _trace=False,
           **trace_kwargs):
    static_enc = np.asarray(static_enc, dtype=np.float16)
    dynamic_enc = np.asarray(dynamic_enc, dtype=np.float16)
    decoder_hidden = np.ascontiguousarray(decoder_hidden, dtype=np.float32)
    v = np.ascontiguousarray(v, dtype=np.float32)
    W = np.ascontiguousarray(W, dtype=np.float32)

    nc = _build()

    xcat = np.concatenate([static_enc, dynamic_enc], axis=1)  # [B, 2H, S]
    wt = _chunk_major(np.concatenate([W[:, :H].T, W[:, H:2 * H].T],
                                     axis=0).astype(np.float16))
    w3t = _chunk_major(np.ascontiguousarray(W[:, 2 * H:].T))
    # vt_ext[p, c, :] = [0]*31 ++ [1] ++ [0]*31 ; lhsT window starting
    # at (HROW-1)-r puts the sum at output row r, zeros elsewhere.
    vt_ext = np.zeros((128, 2, 2 * HROW - 1), dtype=np.float16)
    vt_ext[:, :, HROW - 1] = 1.0  # ones window: plain partition sum
    vp = np.ascontiguousarray(v.reshape(2, 128).T.astype(np.float32))
    # b2[p, r] = 1 iff score rows p and r belong to the same batch (both
    # within the same half); the tot matmul then lands Z_b on every score
    # row partition of batch b directly.
    b2 = np.zeros((128, NROW), dtype=np.float16)
    for r in range(NROW):
        for p in range(NROW):
            if p // NS == r // NS:
                b2[p, r] = 1.0
    in_maps = []
    for i in range(NCORES):
        sl = slice(i * BPC, (i + 1) * BPC)
        ht = _chunk_major(np.ascontiguousarray(decoder_hidden[sl].T))
        in_maps.append({
            "x": xcat[sl],
            "wt": wt, "w3t": w3t, "vt": vt_ext, "ht": ht,
            "b2": b2, "vp": vp,
        })

    res = run_bass_kernel_spmd(nc, in_maps, core_ids=list(range(NCORES)),
                               trace=_trace, **trace_kwargs)
    kernel.last_result = res
    return np.concatenate([res.results[i]["out"] for i in range(NCORES)], axis=0)


kernel.last_result = None
